# revision 2
# baseline (speedup 1.0000x reference)
"""Trainium2 Bass kernel for nn_CausalLSTMCell (6-node causal LSTM cell over
batch 262144).  Self-contained: hardcodes shapes/sharding; host-side numpy
does layout transforms; 8 NeuronCores run an SPMD Tile kernel.

Layout strategy (feature-major): batch on the free dimension, per-node
features (16 each) stacked on partitions.  Dense node order SIGMA =
[0,3,1,4,2,5] puts leaf nodes at partition bases {0,32,64} (legal engine
bases) and internal nodes at {16,48,80} (bridged to base-0 tiles via
SBUF->SBUF DMA, which is partition-unrestricted).  The TF-faithful
child_r reshape scrambles batch rows; each core receives the exact
pre-gate source rows it needs (r4b/r5b buffers) and consumes them with
mod-2/mod-3 residue-class strided column ops.  Shard stride 32766 and
device width 32784 are multiples of 6 so residue phases are identical on
every core (single SPMD program).
"""
import sys
import numpy as np


def _import_concourse():
    for p in ("/opt/trn_rl_repo", "/root/.axon_site/_ro/trn_rl_repo"):
        if p not in sys.path:
            sys.path.insert(0, p)
    import concourse.bacc as bacc  # noqa: F401
    import concourse.mybir as mybir  # noqa: F401
    import concourse.tile as tile  # noqa: F401
    from concourse.bass_utils import run_bass_kernel_spmd  # noqa: F401
    return bacc, mybir, tile, run_bass_kernel_spmd


H = 16
NODES = 6
NCORES = 8
INPUT_IDX = [[0], [1], [2], [0, 3], [1, 4], [2, 5]]
CHILDREN = [0, 0, 0, 1, 2, 3]
SIGMA = [0, 3, 1, 4, 2, 5]          # dense row block j holds node SIGMA[j]
POS = [0, 2, 4, 1, 3, 5]            # node i lives at dense block POS[i]
F_MAIN = 1536

# wblob column offsets
WOFF = {"ig": 0, "fg": 96, "og": 192, "a": 288, "n2": 384,
        "n1r3": 480, "r4": 592, "r5": 656}
WCOLS = 752
# bias columns in bblob
BCOL = {"ig": 0, "fg": 1, "og": 2, "a": 3, "n2": 4, "n1r3": 5, "r4": 6, "r5": 7}


def _plan(B):
    stride = (B // NCORES // 6) * 6
    need = B - (NCORES - 1) * stride
    s_dev = ((need + 5) // 6) * 6
    n_main = s_dev // F_MAIN
    chunks = [F_MAIN] * n_main
    rem = s_dev - n_main * F_MAIN
    if rem:
        chunks.append(rem)
    L4 = s_dev // 2 + 2
    L5 = (2 + s_dev - 1) // 3 + 3
    return stride, s_dev, chunks, L4, L5


def _np(x):
    return np.asarray(x, dtype=np.float32)


def _build_weights(params):
    """wblob [128, 752] fp32 and bblob [128, 8] fp32 (see layout consts)."""
    wblob = np.zeros((128, WCOLS), np.float32)
    bblob = np.zeros((128, 8), np.float32)

    def put_role(dst_off, j, Wx, Wh, bx, bh, idx, i, w16=16):
        for d, xi in enumerate(idx):
            wblob[xi, dst_off + 16 * j: dst_off + 16 * j + w16] += _np(Wx)[d]
        wblob[6 + 16 * i: 6 + 16 * i + 16,
              dst_off + 16 * j: dst_off + 16 * j + w16] = _np(Wh)
        return _np(bx) + _np(bh)

    for j, i in enumerate(SIGMA):
        p = params[i]
        idx = INPUT_IDX[i]
        ifo_x, ifo_h = _np(p["ifo_x"]["W"]), _np(p["ifo_h"]["W"])
        ifo_bx, ifo_bh = _np(p["ifo_x"]["b"]), _np(p["ifo_h"]["b"])
        for role, sl in (("ig", slice(0, 16)), ("fg", slice(16, 32)),
                         ("og", slice(32, 48))):
            b = put_role(WOFF[role], j, ifo_x[:, sl], ifo_h[:, sl],
                         ifo_bx[sl], ifo_bh[sl], idx, i)
            bblob[16 * j:16 * j + 16, BCOL[role]] = b
        b = put_role(WOFF["a"], j, p["a_x"]["W"], p["a_h"]["W"],
                     p["a_x"]["b"], p["a_h"]["b"], idx, i)
        bblob[16 * j:16 * j + 16, BCOL["a"]] = b
        b = put_role(WOFF["n2"], j, p["n2_x"]["W"], p["n2_h"]["W"],
                     p["n2_x"]["b"], p["n2_h"]["b"], idx, i)
        bblob[16 * j:16 * j + 16, BCOL["n2"]] = b

    # n1r3 [102, 112]: r3@0, n1(node4)@32, n1(node5)@64, n1(node3)@96
    bblob[0:112, BCOL["n1r3"]] = -40.0
    p3 = params[3]
    for d, xi in enumerate(INPUT_IDX[3]):
        wblob[xi, WOFF["n1r3"] + 0: WOFF["n1r3"] + 16] += _np(p3["r_x"]["W"])[d]
    wblob[6 + 48:6 + 64, WOFF["n1r3"] + 0: WOFF["n1r3"] + 16] = _np(p3["r_h"]["W"])
    bblob[0:16, BCOL["n1r3"]] = _np(p3["r_x"]["b"]) + _np(p3["r_h"]["b"])
    for node_i, coff in ((4, 32), (5, 64), (3, 96)):
        p = params[node_i]
        for d, xi in enumerate(INPUT_IDX[node_i]):
            wblob[xi, WOFF["n1r3"] + coff: WOFF["n1r3"] + coff + 16] += \
                _np(p["n1_x"]["W"])[d]
        wblob[6 + 16 * node_i:6 + 16 * node_i + 16,
              WOFF["n1r3"] + coff: WOFF["n1r3"] + coff + 16] = _np(p["n1_h"]["W"])
        bblob[coff:coff + 16, BCOL["n1r3"]] = \
            _np(p["n1_x"]["b"]) + _np(p["n1_h"]["b"])

    # child weights: rows 0:2 = x cols INPUT_IDX[i]; rows 2:18 = h_i
    bblob[0:64, BCOL["r4"]] = -40.0
    bblob[0:96, BCOL["r5"]] = -40.0
    for node_i, key, nc_i in ((4, "r4", 2), (5, "r5", 3)):
        p = params[node_i]
        Wx, Wh = _np(p["r_x"]["W"]), _np(p["r_h"]["W"])
        bb = _np(p["r_x"]["b"]) + _np(p["r_h"]["b"])
        for g in range(nc_i):
            co = WOFF[key] + 32 * g
            wblob[0:2, co:co + 16] = Wx[:, 16 * g:16 * g + 16]
            wblob[2:18, co:co + 16] = Wh[:, 16 * g:16 * g + 16]
            bblob[32 * g:32 * g + 16, BCOL[key]] = bb[16 * g:16 * g + 16]
    return wblob, bblob


def _host_prep_core(x, h, c, cix, B, stride, s_dev, L4, L5):
    b0 = cix * stride
    cols = np.arange(b0, b0 + s_dev)
    valid = cols < B
    colsc = np.minimum(cols, B - 1)
    xh = np.zeros((102, s_dev), np.float32)
    xh[0:6, :] = np.where(valid, x[colsc, :].T, 0.0)
    for i in range(NODES):
        xh[6 + 16 * i:6 + 16 * i + 16, :] = np.where(valid, h[i, colsc, :].T, 0.0)
    c6 = np.zeros((96, s_dev), np.float32)
    for j, i in enumerate(SIGMA):
        c6[16 * j:16 * j + 16, :] = np.where(valid, c[i, colsc, :].T, 0.0)
    r4b = np.zeros((2, 18, L4), np.float32)
    for k in range(2):
        t0 = k * B + b0
        assert t0 % 2 == 0
        rho0 = t0 // 2
        rows = np.arange(rho0, rho0 + L4)
        rv = rows < B
        rc = np.minimum(rows, B - 1)
        r4b[k, 0:2, :] = np.where(rv, x[rc][:, INPUT_IDX[4]].T, 0.0)
        r4b[k, 2:18, :] = np.where(rv, h[4, rc, :].T, 0.0)
    r5b = np.zeros((3, 18, L5), np.float32)
    for k in range(3):
        t0 = k * B + b0
        delta = t0 % 3
        rho0 = (t0 - delta) // 3
        Lk = (delta + s_dev - 1) // 3 + 1
        assert Lk <= L5
        rows = np.arange(rho0, rho0 + Lk)
        rv = rows < B
        rc = np.minimum(rows, B - 1)
        r5b[k, 0:2, :Lk] = np.where(rv, x[rc][:, INPUT_IDX[5]].T, 0.0)
        r5b[k, 2:18, :Lk] = np.where(rv, h[5, rc, :].T, 0.0)
    return xh, c6, r4b, r5b


def _pieces(F):
    out = []
    p = 0
    while p < F:
        w = min(512, F - p)
        out.append((p, w))
        p += w
    return out


def _build_program(bacc, mybir, tile, s_dev, chunks, L4, L5):
    f32 = mybir.dt.float32
    f32r = mybir.dt.float32r
    Sig = mybir.ActivationFunctionType.Sigmoid
    Tanh = mybir.ActivationFunctionType.Tanh
    MUL = mybir.AluOpType.mult
    ADD = mybir.AluOpType.add

    nc = bacc.Bacc("TRN2", target_bir_lowering=False, debug=False,
                   num_devices=NCORES)
    xh_d = nc.dram_tensor("xh", [102, s_dev], f32r, kind="ExternalInput")
    c6_d = nc.dram_tensor("c6", [96, s_dev], f32, kind="ExternalInput")
    r4_d = nc.dram_tensor("r4b", [2, 18, L4], f32r, kind="ExternalInput")
    r5_d = nc.dram_tensor("r5b", [3, 18, L5], f32r, kind="ExternalInput")
    wb_d = nc.dram_tensor("wblob", [128, WCOLS], f32r, kind="ExternalInput")
    bb_d = nc.dram_tensor("bblob", [128, 8], f32, kind="ExternalInput")
    nt_d = nc.dram_tensor("nt", [96, s_dev], f32, kind="ExternalOutput")
    ht_d = nc.dram_tensor("ht", [96, s_dev], f32, kind="ExternalOutput")
    ct_d = nc.dram_tensor("ct", [96, s_dev], f32, kind="ExternalOutput")

    def _even(v):
        return v + (v % 2)

    FM = chunks[0]
    FHM, F3M = _even(FM // 2), _even(FM // 3 + 1)

    with tile.TileContext(nc) as tc:
        with tc.tile_pool(name="const", bufs=1) as cpool, \
             tc.tile_pool(name="io", bufs=2) as io, \
             tc.tile_pool(name="sig", bufs=1) as sg, \
             tc.tile_pool(name="work", bufs=2) as wk, \
             tc.tile_pool(name="narrow", bufs=1) as nr, \
             tc.tile_pool(name="st", bufs=2, space="PSUM") as psr, \
             tc.tile_pool(name="stc", bufs=1, space="PSUM") as psc:

            wb = cpool.tile([128, WCOLS], f32r)
            bb = cpool.tile([128, 8], f32)
            nc.sync.dma_start(out=wb, in_=wb_d[:, :])
            nc.sync.dma_start(out=bb, in_=bb_d[:, :])

            def role_matmul_act(XH, F, woff, rows, func, bcol, dst):
                st = psr.tile([112, FM], f32, tag="st")
                for p0, w in _pieces(F):
                    nc.tensor.matmul(st[0:rows, p0:p0 + w],
                                     wb[0:102, woff:woff + rows],
                                     XH[:, p0:p0 + w], start=True, stop=True)
                nc.scalar.activation(dst[:, 0:F], st[0:rows, 0:F], func,
                                     bias=bb[0:rows, bcol:bcol + 1], scale=1.0)

            c0 = 0
            for F in chunks:
                FH, F3, FT = _even(F // 2), _even(F // 3 + 1), F // 3
                XH = io.tile([102, FM], f32r, tag="xh")
                nc.sync.dma_start(out=XH[:, 0:F], in_=xh_d[:, c0:c0 + F])
                CP = io.tile([96, FM], f32, tag="cp")
                nc.sync.dma_start(out=CP[:, 0:F], in_=c6_d[:, c0:c0 + F])
                R4 = []
                for k in range(2):
                    t = io.tile([18, FHM], f32r, tag=f"r4_{k}")
                    nc.sync.dma_start(out=t[:, 0:FH],
                                      in_=r4_d[k, :, c0 // 2:c0 // 2 + FH])
                    R4.append(t)
                R5 = []
                for k in range(3):
                    t = io.tile([18, F3M], f32r, tag=f"r5_{k}")
                    nc.sync.dma_start(out=t[:, 0:F3],
                                      in_=r5_d[k, :, c0 // 3:c0 // 3 + F3])
                    R5.append(t)

                IG = sg.tile([96, FM], f32, tag="ig")
                FG = sg.tile([96, FM], f32, tag="fg")
                OG = sg.tile([96, FM], f32, tag="og")
                AT = sg.tile([96, FM], f32, tag="at")
                N2 = sg.tile([96, FM], f32, tag="n2")
                N1R = sg.tile([112, FM], f32, tag="n1r")
                role_matmul_act(XH, F, WOFF["ig"], 96, Sig, BCOL["ig"], IG)
                role_matmul_act(XH, F, WOFF["fg"], 96, Sig, BCOL["fg"], FG)
                role_matmul_act(XH, F, WOFF["og"], 96, Sig, BCOL["og"], OG)
                role_matmul_act(XH, F, WOFF["a"], 96, Tanh, BCOL["a"], AT)
                role_matmul_act(XH, F, WOFF["n2"], 96, Sig, BCOL["n2"], N2)
                role_matmul_act(XH, F, WOFF["n1r3"], 112, Sig, BCOL["n1r3"], N1R)

                # child pre-gates: sigmoid in place in PSUM
                CR4 = []
                for k in range(2):
                    st = psc.tile([96, max(FHM, F3M)], f32, tag="stc")
                    for p0, w in _pieces(FH):
                        nc.tensor.matmul(st[0:64, p0:p0 + w],
                                         wb[0:18, WOFF["r4"]:WOFF["r4"] + 64],
                                         R4[k][:, p0:p0 + w],
                                         start=True, stop=True)
                    nc.scalar.activation(st[0:64, 0:FH], st[0:64, 0:FH], Sig,
                                         bias=bb[0:64, BCOL["r4"]:BCOL["r4"] + 1],
                                         scale=1.0)
                    CR4.append(st)
                CR5 = []
                for k in range(3):
                    st = psc.tile([96, max(FHM, F3M)], f32, tag="stc")
                    for p0, w in _pieces(F3):
                        nc.tensor.matmul(st[0:96, p0:p0 + w],
                                         wb[0:18, WOFF["r5"]:WOFF["r5"] + 96],
                                         R5[k][:, p0:p0 + w],
                                         start=True, stop=True)
                    nc.scalar.activation(st[0:96, 0:F3], st[0:96, 0:F3], Sig,
                                         bias=bb[0:96, BCOL["r5"]:BCOL["r5"] + 1],
                                         scale=1.0)
                    CR5.append(st)

                # horizontal pass (dense sigma order)
                CN = wk.tile([96, FM], f32, tag="cn")
                T2 = wk.tile([96, FM], f32, tag="t2")
                TC = wk.tile([96, FM], f32, tag="tc")
                HN = wk.tile([96, FM], f32, tag="hn")
                NT = wk.tile([96, FM], f32, tag="nt")
                nc.vector.tensor_tensor(CN[:, 0:F], IG[:, 0:F], AT[:, 0:F], MUL)
                nc.vector.tensor_tensor(T2[:, 0:F], FG[:, 0:F], CP[:, 0:F], MUL)
                nc.vector.tensor_tensor(CN[:, 0:F], CN[:, 0:F], T2[:, 0:F], ADD)
                nc.scalar.activation(TC[:, 0:F], CN[:, 0:F], Tanh, scale=1.0)
                nc.vector.tensor_tensor(HN[:, 0:F], OG[:, 0:F], TC[:, 0:F], MUL)
                nc.vector.tensor_tensor(NT[:, 0:F], N2[:, 0:F], HN[:, 0:F], MUL)

                # narrow tiles (see module docstring for slot/base plan)
                SM1 = nr.tile([128, FM], f32, tag="sm1")
                SM2 = nr.tile([128, FM], f32, tag="sm2")
                SM3 = nr.tile([128, FM], f32, tag="sm3")
                SM4 = nr.tile([64, FM], f32, tag="sm4")
                SM5 = nr.tile([64, FM], f32, tag="sm5")
                ntp3, p40, p50, p30 = SM1[0:16], SM1[32:48], SM1[64:80], SM1[96:112]
                T16a, p41, p51 = SM2[0:16], SM2[32:48], SM2[64:80]
                ntp4, p52 = SM3[0:16], SM3[64:80]
                T16b, ntp5 = SM4[0:16], SM4[32:48]
                T16c = SM5[32:48]

                # bridges: internal-node prelim n_new blocks to base-0/32 slots
                nc.sync.dma_start(out=ntp3[:, 0:F], in_=NT[16:32, 0:F])
                nc.sync.dma_start(out=ntp4[:, 0:F], in_=NT[48:64, 0:F])
                nc.sync.dma_start(out=ntp5[:, 0:F], in_=NT[80:96, 0:F])

                # node 3 (b-aligned child): r3 = sig(...) at N1R[0:16]
                nc.vector.tensor_tensor(p30[:, 0:F], N1R[0:16, 0:F],
                                        NT[0:16, 0:F], MUL)
                nc.vector.tensor_tensor(T16a[:, 0:F], N1R[96:112, 0:F],
                                        p30[:, 0:F], MUL)
                nc.gpsimd.tensor_tensor(ntp3[:, 0:F], ntp3[:, 0:F],
                                        T16a[:, 0:F], ADD)

                # node 4: classes mod 2; children n1 (dense@32), n2 (dense@64)
                for k, (pdst, nsrc) in enumerate(((p40, NT[32:48]),
                                                  (p41, NT[64:80]))):
                    for m2 in range(2):
                        nc.vector.tensor_tensor(
                            pdst[:, m2:F:2],
                            CR4[k][32 * m2:32 * m2 + 16, 0:FH],
                            nsrc[:, m2:F:2], MUL)
                nc.gpsimd.tensor_tensor(p40[:, 0:F], p40[:, 0:F],
                                        p41[:, 0:F], ADD)
                nc.vector.tensor_tensor(T16b[:, 0:F], N1R[32:48, 0:F],
                                        p40[:, 0:F], MUL)
                nc.gpsimd.tensor_tensor(ntp4[:, 0:F], ntp4[:, 0:F],
                                        T16b[:, 0:F], ADD)

                # node 5: classes mod 3; children n0 (dense@0), n3f=ntp3, n4f=ntp4
                for k, (pdst, nsrc) in enumerate(((p50, NT[0:16]),
                                                  (p51, ntp3),
                                                  (p52, ntp4))):
                    for m3 in range(3):
                        g = (k + m3) % 3
                        joff = (k + m3) // 3
                        nc.vector.tensor_tensor(
                            pdst[:, m3:F:3],
                            CR5[k][32 * g:32 * g + 16, joff:joff + FT],
                            nsrc[:, m3:F:3], MUL)
                nc.gpsimd.tensor_tensor(p50[:, 0:F], p50[:, 0:F],
                                        p51[:, 0:F], ADD)
                nc.gpsimd.tensor_tensor(p50[:, 0:F], p50[:, 0:F],
                                        p52[:, 0:F], ADD)
                nc.vector.tensor_tensor(T16c[:, 0:F], N1R[64:80, 0:F],
                                        p50[:, 0:F], MUL)
                nc.gpsimd.tensor_tensor(ntp5[:, 0:F], ntp5[:, 0:F],
                                        T16c[:, 0:F], ADD)

                # outputs
                nc.sync.dma_start(out=ht_d[:, c0:c0 + F], in_=HN[:, 0:F])
                nc.sync.dma_start(out=ct_d[:, c0:c0 + F], in_=CN[:, 0:F])
                nc.sync.dma_start(out=nt_d[0:16, c0:c0 + F], in_=NT[0:16, 0:F])
                nc.sync.dma_start(out=nt_d[32:48, c0:c0 + F], in_=NT[32:48, 0:F])
                nc.sync.dma_start(out=nt_d[64:80, c0:c0 + F], in_=NT[64:80, 0:F])
                nc.sync.dma_start(out=nt_d[16:32, c0:c0 + F], in_=ntp3[:, 0:F])
                nc.sync.dma_start(out=nt_d[48:64, c0:c0 + F], in_=ntp4[:, 0:F])
                nc.sync.dma_start(out=nt_d[80:96, c0:c0 + F], in_=ntp5[:, 0:F])
                c0 += F

    nc.finalize()
    return nc


def kernel(inputs, h, c, n, params):
    bacc, mybir, tile, run_bass_kernel_spmd = _import_concourse()
    x = _np(inputs)
    h = _np(h)
    c = _np(c)
    B = x.shape[0]
    stride, s_dev, chunks, L4, L5 = _plan(B)
    wblob, bblob = _build_weights(params)
    in_maps = []
    for cix in range(NCORES):
        xh, c6, r4b, r5b = _host_prep_core(x, h, c, cix, B, stride, s_dev, L4, L5)
        in_maps.append(dict(xh=xh, c6=c6, r4b=r4b, r5b=r5b,
                            wblob=wblob, bblob=bblob))
    nc = _build_program(bacc, mybir, tile, s_dev, chunks, L4, L5)
    res = run_bass_kernel_spmd(nc, in_maps, list(range(NCORES))).results

    n_out = np.empty((NODES, B, H), np.float32)
    h_out = np.empty((NODES, B, H), np.float32)
    c_out = np.empty((NODES, B, H), np.float32)
    for cix in range(NCORES):
        b0 = cix * stride
        w = min(s_dev, B - b0)
        for j, i in enumerate(SIGMA):
            n_out[i, b0:b0 + w] = res[cix]["nt"][16 * j:16 * j + 16, :w].T
            h_out[i, b0:b0 + w] = res[cix]["ht"][16 * j:16 * j + 16, :w].T
            c_out[i, b0:b0 + w] = res[cix]["ct"][16 * j:16 * j + 16, :w].T
    return n_out, h_out, c_out


# revision 4
# speedup vs baseline: 1.2104x; 1.2104x over previous
"""Trainium2 Bass kernel for nn_CausalLSTMCell (6-node causal LSTM cell over
batch 262144).  Self-contained: hardcodes shapes/sharding; host-side numpy
does layout transforms; 8 NeuronCores run an SPMD Tile kernel.

Layout strategy (feature-major): batch on the free dimension, per-node
features (16 each) stacked on partitions.  Dense node order SIGMA =
[0,3,1,4,2,5] puts leaf nodes at partition bases {0,32,64} (legal engine
bases) and internal nodes at {16,48,80} (bridged to base-0 tiles via
SBUF->SBUF DMA, which is partition-unrestricted).  The TF-faithful
child_r reshape scrambles batch rows; each core receives the exact
pre-gate source rows it needs (r4b/r5b buffers) and consumes them with
mod-2/mod-3 residue-class strided column ops.  Shard stride 32766 and
device width 32784 are multiples of 6 so residue phases are identical on
every core (single SPMD program).
"""
import sys
import numpy as np


def _import_concourse():
    for p in ("/opt/trn_rl_repo", "/root/.axon_site/_ro/trn_rl_repo"):
        if p not in sys.path:
            sys.path.insert(0, p)
    import concourse.bacc as bacc  # noqa: F401
    import concourse.mybir as mybir  # noqa: F401
    import concourse.tile as tile  # noqa: F401
    from concourse.bass_utils import run_bass_kernel_spmd  # noqa: F401
    return bacc, mybir, tile, run_bass_kernel_spmd


H = 16
NODES = 6
NCORES = 8
INPUT_IDX = [[0], [1], [2], [0, 3], [1, 4], [2, 5]]
CHILDREN = [0, 0, 0, 1, 2, 3]
SIGMA = [0, 3, 1, 4, 2, 5]          # dense row block j holds node SIGMA[j]
POS = [0, 2, 4, 1, 3, 5]            # node i lives at dense block POS[i]
F_MAIN = 1536

# wblob column offsets
WOFF = {"ig": 0, "fg": 96, "og": 192, "a": 288, "n2": 384,
        "n1r3": 480, "r4": 592, "r5": 656}
WCOLS = 752
# bias columns in bblob
BCOL = {"ig": 0, "fg": 1, "og": 2, "a": 3, "n2": 4, "n1r3": 5, "r4": 6, "r5": 7}


def _plan(B):
    stride = (B // NCORES // 6) * 6
    need = B - (NCORES - 1) * stride
    s_dev = ((need + 5) // 6) * 6
    n_main = s_dev // F_MAIN
    chunks = [F_MAIN] * n_main
    rem = s_dev - n_main * F_MAIN
    if rem:
        chunks.append(rem)
    L4 = s_dev // 2 + 2
    L5 = (2 + s_dev - 1) // 3 + 3
    return stride, s_dev, chunks, L4, L5


def _np(x):
    return np.asarray(x, dtype=np.float32)


def _build_weights(params):
    """wblob [128, 752] fp32 and bblob [128, 8] fp32 (see layout consts)."""
    wblob = np.zeros((128, WCOLS), np.float32)
    bblob = np.zeros((128, 8), np.float32)

    def put_role(dst_off, j, Wx, Wh, bx, bh, idx, i, w16=16):
        for d, xi in enumerate(idx):
            wblob[xi, dst_off + 16 * j: dst_off + 16 * j + w16] += _np(Wx)[d]
        wblob[6 + 16 * i: 6 + 16 * i + 16,
              dst_off + 16 * j: dst_off + 16 * j + w16] = _np(Wh)
        return _np(bx) + _np(bh)

    for j, i in enumerate(SIGMA):
        p = params[i]
        idx = INPUT_IDX[i]
        ifo_x, ifo_h = _np(p["ifo_x"]["W"]), _np(p["ifo_h"]["W"])
        ifo_bx, ifo_bh = _np(p["ifo_x"]["b"]), _np(p["ifo_h"]["b"])
        for role, sl in (("ig", slice(0, 16)), ("fg", slice(16, 32)),
                         ("og", slice(32, 48))):
            b = put_role(WOFF[role], j, ifo_x[:, sl], ifo_h[:, sl],
                         ifo_bx[sl], ifo_bh[sl], idx, i)
            bblob[16 * j:16 * j + 16, BCOL[role]] = b
        b = put_role(WOFF["a"], j, p["a_x"]["W"], p["a_h"]["W"],
                     p["a_x"]["b"], p["a_h"]["b"], idx, i)
        bblob[16 * j:16 * j + 16, BCOL["a"]] = b
        b = put_role(WOFF["n2"], j, p["n2_x"]["W"], p["n2_h"]["W"],
                     p["n2_x"]["b"], p["n2_h"]["b"], idx, i)
        bblob[16 * j:16 * j + 16, BCOL["n2"]] = b

    # n1r3 [102, 112]: r3@0, n1(node4)@32, n1(node5)@64, n1(node3)@96
    bblob[0:112, BCOL["n1r3"]] = -40.0
    p3 = params[3]
    for d, xi in enumerate(INPUT_IDX[3]):
        wblob[xi, WOFF["n1r3"] + 0: WOFF["n1r3"] + 16] += _np(p3["r_x"]["W"])[d]
    wblob[6 + 48:6 + 64, WOFF["n1r3"] + 0: WOFF["n1r3"] + 16] = _np(p3["r_h"]["W"])
    bblob[0:16, BCOL["n1r3"]] = _np(p3["r_x"]["b"]) + _np(p3["r_h"]["b"])
    for node_i, coff in ((4, 32), (5, 64), (3, 96)):
        p = params[node_i]
        for d, xi in enumerate(INPUT_IDX[node_i]):
            wblob[xi, WOFF["n1r3"] + coff: WOFF["n1r3"] + coff + 16] += \
                _np(p["n1_x"]["W"])[d]
        wblob[6 + 16 * node_i:6 + 16 * node_i + 16,
              WOFF["n1r3"] + coff: WOFF["n1r3"] + coff + 16] = _np(p["n1_h"]["W"])
        bblob[coff:coff + 16, BCOL["n1r3"]] = \
            _np(p["n1_x"]["b"]) + _np(p["n1_h"]["b"])

    # child weights: rows 0:2 = x cols INPUT_IDX[i]; rows 2:18 = h_i
    bblob[0:64, BCOL["r4"]] = -40.0
    bblob[0:96, BCOL["r5"]] = -40.0
    for node_i, key, nc_i in ((4, "r4", 2), (5, "r5", 3)):
        p = params[node_i]
        Wx, Wh = _np(p["r_x"]["W"]), _np(p["r_h"]["W"])
        bb = _np(p["r_x"]["b"]) + _np(p["r_h"]["b"])
        for g in range(nc_i):
            co = WOFF[key] + 32 * g
            wblob[0:2, co:co + 16] = Wx[:, 16 * g:16 * g + 16]
            wblob[2:18, co:co + 16] = Wh[:, 16 * g:16 * g + 16]
            bblob[32 * g:32 * g + 16, BCOL[key]] = bb[16 * g:16 * g + 16]
    return wblob, bblob


def _host_prep_core(x, h, c, cix, B, stride, s_dev, L4, L5):
    b0 = cix * stride
    cols = np.arange(b0, b0 + s_dev)
    valid = cols < B
    colsc = np.minimum(cols, B - 1)
    xh = np.zeros((102, s_dev), np.float32)
    xh[0:6, :] = np.where(valid, x[colsc, :].T, 0.0)
    for i in range(NODES):
        xh[6 + 16 * i:6 + 16 * i + 16, :] = np.where(valid, h[i, colsc, :].T, 0.0)
    c6 = np.zeros((96, s_dev), np.float16)
    for j, i in enumerate(SIGMA):
        c6[16 * j:16 * j + 16, :] = np.where(valid, c[i, colsc, :].T, 0.0).astype(np.float16)
    r4b = np.zeros((2, 18, L4), np.float32)
    for k in range(2):
        t0 = k * B + b0
        assert t0 % 2 == 0
        rho0 = t0 // 2
        rows = np.arange(rho0, rho0 + L4)
        rv = rows < B
        rc = np.minimum(rows, B - 1)
        r4b[k, 0:2, :] = np.where(rv, x[rc][:, INPUT_IDX[4]].T, 0.0)
        r4b[k, 2:18, :] = np.where(rv, h[4, rc, :].T, 0.0)
    r5b = np.zeros((3, 18, L5), np.float32)
    for k in range(3):
        t0 = k * B + b0
        delta = t0 % 3
        rho0 = (t0 - delta) // 3
        Lk = (delta + s_dev - 1) // 3 + 1
        assert Lk <= L5
        rows = np.arange(rho0, rho0 + Lk)
        rv = rows < B
        rc = np.minimum(rows, B - 1)
        r5b[k, 0:2, :Lk] = np.where(rv, x[rc][:, INPUT_IDX[5]].T, 0.0)
        r5b[k, 2:18, :Lk] = np.where(rv, h[5, rc, :].T, 0.0)
    return xh, c6, r4b, r5b


def _pieces(F):
    out = []
    p = 0
    while p < F:
        w = min(512, F - p)
        out.append((p, w))
        p += w
    return out


def _build_program(bacc, mybir, tile, s_dev, chunks, L4, L5):
    f32 = mybir.dt.float32
    f32r = mybir.dt.float32r
    bf16 = mybir.dt.float16
    Sig = mybir.ActivationFunctionType.Sigmoid
    Tanh = mybir.ActivationFunctionType.Tanh
    MUL = mybir.AluOpType.mult
    ADD = mybir.AluOpType.add

    nc = bacc.Bacc("TRN2", target_bir_lowering=False, debug=False,
                   num_devices=NCORES)
    xh_d = nc.dram_tensor("xh", [102, s_dev], f32r, kind="ExternalInput")
    c6_d = nc.dram_tensor("c6", [96, s_dev], bf16, kind="ExternalInput")
    r4_d = nc.dram_tensor("r4b", [2, 18, L4], f32r, kind="ExternalInput")
    r5_d = nc.dram_tensor("r5b", [3, 18, L5], f32r, kind="ExternalInput")
    wb_d = nc.dram_tensor("wblob", [128, WCOLS], f32r, kind="ExternalInput")
    bb_d = nc.dram_tensor("bblob", [128, 8], f32, kind="ExternalInput")
    nt_d = nc.dram_tensor("nt", [96, s_dev], bf16, kind="ExternalOutput")
    ht_d = nc.dram_tensor("ht", [96, s_dev], bf16, kind="ExternalOutput")
    ct_d = nc.dram_tensor("ct", [96, s_dev], bf16, kind="ExternalOutput")

    def _even(v):
        return v + (v % 2)

    FM = chunks[0]
    FHM, F3M = _even(FM // 2), _even(FM // 3 + 1)

    with tile.TileContext(nc) as tc:
        with tc.tile_pool(name="const", bufs=1) as cpool, \
             tc.tile_pool(name="io", bufs=3) as io, \
             tc.tile_pool(name="sig", bufs=2) as sg, \
             tc.tile_pool(name="work", bufs=2) as wk, \
             tc.tile_pool(name="narrow", bufs=2) as nr, \
             tc.tile_pool(name="st", bufs=2, space="PSUM") as psr, \
             tc.tile_pool(name="stc", bufs=1, space="PSUM") as psc:

            wb = cpool.tile([128, WCOLS], f32r)
            bb = cpool.tile([128, 8], f32)
            nc.sync.dma_start(out=wb, in_=wb_d[:, :])
            nc.sync.dma_start(out=bb, in_=bb_d[:, :])

            def role_matmul_act(XH, F, woff, rows, func, bcol, dst):
                st = psr.tile([112, FM], f32, tag="st")
                for p0, w in _pieces(F):
                    nc.tensor.matmul(st[0:rows, p0:p0 + w],
                                     wb[0:102, woff:woff + rows],
                                     XH[:, p0:p0 + w], start=True, stop=True)
                nc.scalar.activation(dst[:, 0:F], st[0:rows, 0:F], func,
                                     bias=bb[0:rows, bcol:bcol + 1], scale=1.0)

            c0 = 0
            for F in chunks:
                FH, F3, FT = _even(F // 2), _even(F // 3 + 1), F // 3
                XH = io.tile([102, FM], f32r, tag="xh")
                nc.sync.dma_start(out=XH[:, 0:F], in_=xh_d[:, c0:c0 + F])
                CP = io.tile([96, FM], bf16, tag="cp")
                nc.sync.dma_start(out=CP[:, 0:F], in_=c6_d[:, c0:c0 + F])
                R4 = []
                for k in range(2):
                    t = io.tile([18, FHM], f32r, tag=f"r4_{k}")
                    nc.sync.dma_start(out=t[:, 0:FH],
                                      in_=r4_d[k, :, c0 // 2:c0 // 2 + FH])
                    R4.append(t)
                R5 = []
                for k in range(3):
                    t = io.tile([18, F3M], f32r, tag=f"r5_{k}")
                    nc.sync.dma_start(out=t[:, 0:F3],
                                      in_=r5_d[k, :, c0 // 3:c0 // 3 + F3])
                    R5.append(t)

                IG = sg.tile([96, FM], bf16, tag="ig")
                FG = sg.tile([96, FM], bf16, tag="fg")
                OG = sg.tile([96, FM], bf16, tag="og")
                AT = sg.tile([96, FM], bf16, tag="at")
                N2 = sg.tile([96, FM], bf16, tag="n2")
                N1R = sg.tile([112, FM], bf16, tag="n1r")
                role_matmul_act(XH, F, WOFF["ig"], 96, Sig, BCOL["ig"], IG)
                role_matmul_act(XH, F, WOFF["fg"], 96, Sig, BCOL["fg"], FG)
                role_matmul_act(XH, F, WOFF["og"], 96, Sig, BCOL["og"], OG)
                role_matmul_act(XH, F, WOFF["a"], 96, Tanh, BCOL["a"], AT)
                role_matmul_act(XH, F, WOFF["n2"], 96, Sig, BCOL["n2"], N2)
                role_matmul_act(XH, F, WOFF["n1r3"], 112, Sig, BCOL["n1r3"], N1R)

                # child pre-gates: sigmoid in place in PSUM
                CR4 = []
                for k in range(2):
                    st = psc.tile([96, max(FHM, F3M)], f32, tag="stc")
                    for p0, w in _pieces(FH):
                        nc.tensor.matmul(st[0:64, p0:p0 + w],
                                         wb[0:18, WOFF["r4"]:WOFF["r4"] + 64],
                                         R4[k][:, p0:p0 + w],
                                         start=True, stop=True)
                    nc.scalar.activation(st[0:64, 0:FH], st[0:64, 0:FH], Sig,
                                         bias=bb[0:64, BCOL["r4"]:BCOL["r4"] + 1],
                                         scale=1.0)
                    CR4.append(st)
                CR5 = []
                for k in range(3):
                    st = psc.tile([96, max(FHM, F3M)], f32, tag="stc")
                    for p0, w in _pieces(F3):
                        nc.tensor.matmul(st[0:96, p0:p0 + w],
                                         wb[0:18, WOFF["r5"]:WOFF["r5"] + 96],
                                         R5[k][:, p0:p0 + w],
                                         start=True, stop=True)
                    nc.scalar.activation(st[0:96, 0:F3], st[0:96, 0:F3], Sig,
                                         bias=bb[0:96, BCOL["r5"]:BCOL["r5"] + 1],
                                         scale=1.0)
                    CR5.append(st)

                # horizontal pass (dense sigma order)
                CN = wk.tile([96, FM], bf16, tag="cn")
                T2 = wk.tile([96, FM], bf16, tag="t2")
                TC = wk.tile([96, FM], bf16, tag="tc")
                HN = wk.tile([96, FM], bf16, tag="hn")
                NT = wk.tile([96, FM], bf16, tag="nt")
                nc.vector.tensor_tensor(CN[:, 0:F], IG[:, 0:F], AT[:, 0:F], MUL)
                nc.vector.tensor_tensor(T2[:, 0:F], FG[:, 0:F], CP[:, 0:F], MUL)
                nc.vector.tensor_tensor(CN[:, 0:F], CN[:, 0:F], T2[:, 0:F], ADD)
                nc.scalar.activation(TC[:, 0:F], CN[:, 0:F], Tanh, scale=1.0)
                nc.vector.tensor_tensor(HN[:, 0:F], OG[:, 0:F], TC[:, 0:F], MUL)
                nc.vector.tensor_tensor(NT[:, 0:F], N2[:, 0:F], HN[:, 0:F], MUL)

                # narrow tiles (see module docstring for slot/base plan)
                SM1 = nr.tile([128, FM], bf16, tag="sm1")
                SM2 = nr.tile([128, FM], bf16, tag="sm2")
                SM3 = nr.tile([128, FM], bf16, tag="sm3")
                SM4 = nr.tile([64, FM], bf16, tag="sm4")
                SM5 = nr.tile([64, FM], bf16, tag="sm5")
                ntp3, p40, p50, p30 = SM1[0:16], SM1[32:48], SM1[64:80], SM1[96:112]
                T16a, p41, p51 = SM2[0:16], SM2[32:48], SM2[64:80]
                ntp4, p52 = SM3[0:16], SM3[64:80]
                T16b, ntp5 = SM4[0:16], SM4[32:48]
                T16c = SM5[32:48]

                # bridges: internal-node prelim n_new blocks to base-0/32 slots
                nc.gpsimd.dma_start(out=ntp3[:, 0:F], in_=NT[16:32, 0:F])
                nc.gpsimd.dma_start(out=ntp4[:, 0:F], in_=NT[48:64, 0:F])
                nc.gpsimd.dma_start(out=ntp5[:, 0:F], in_=NT[80:96, 0:F])

                # node 3 (b-aligned child): r3 = sig(...) at N1R[0:16]
                nc.vector.tensor_tensor(p30[:, 0:F], N1R[0:16, 0:F],
                                        NT[0:16, 0:F], MUL)
                nc.gpsimd.tensor_tensor(T16a[:, 0:F], N1R[96:112, 0:F],
                                        p30[:, 0:F], MUL)
                nc.gpsimd.tensor_tensor(ntp3[:, 0:F], ntp3[:, 0:F],
                                        T16a[:, 0:F], ADD)

                # node 4: classes mod 2; children n1 (dense@32), n2 (dense@64)
                for k, (pdst, nsrc) in enumerate(((p40, NT[32:48]),
                                                  (p41, NT[64:80]))):
                    for m2 in range(2):
                        nc.vector.tensor_tensor(
                            pdst[:, m2:F:2],
                            CR4[k][32 * m2:32 * m2 + 16, 0:FH],
                            nsrc[:, m2:F:2], MUL)
                nc.gpsimd.tensor_tensor(p40[:, 0:F], p40[:, 0:F],
                                        p41[:, 0:F], ADD)
                nc.gpsimd.tensor_tensor(T16b[:, 0:F], N1R[32:48, 0:F],
                                        p40[:, 0:F], MUL)
                nc.gpsimd.tensor_tensor(ntp4[:, 0:F], ntp4[:, 0:F],
                                        T16b[:, 0:F], ADD)

                # node 5: classes mod 3; children n0 (dense@0), n3f=ntp3, n4f=ntp4
                for k, (pdst, nsrc) in enumerate(((p50, NT[0:16]),
                                                  (p51, ntp3),
                                                  (p52, ntp4))):
                    for m3 in range(3):
                        g = (k + m3) % 3
                        joff = (k + m3) // 3
                        nc.vector.tensor_tensor(
                            pdst[:, m3:F:3],
                            CR5[k][32 * g:32 * g + 16, joff:joff + FT],
                            nsrc[:, m3:F:3], MUL)
                nc.gpsimd.tensor_tensor(p50[:, 0:F], p50[:, 0:F],
                                        p51[:, 0:F], ADD)
                nc.gpsimd.tensor_tensor(p50[:, 0:F], p50[:, 0:F],
                                        p52[:, 0:F], ADD)
                nc.gpsimd.tensor_tensor(T16c[:, 0:F], N1R[64:80, 0:F],
                                        p50[:, 0:F], MUL)
                nc.gpsimd.tensor_tensor(ntp5[:, 0:F], ntp5[:, 0:F],
                                        T16c[:, 0:F], ADD)

                # outputs
                nc.scalar.dma_start(out=ht_d[:, c0:c0 + F], in_=HN[:, 0:F])
                nc.scalar.dma_start(out=ct_d[:, c0:c0 + F], in_=CN[:, 0:F])
                nc.scalar.dma_start(out=nt_d[0:16, c0:c0 + F], in_=NT[0:16, 0:F])
                nc.scalar.dma_start(out=nt_d[32:48, c0:c0 + F], in_=NT[32:48, 0:F])
                nc.scalar.dma_start(out=nt_d[64:80, c0:c0 + F], in_=NT[64:80, 0:F])
                nc.scalar.dma_start(out=nt_d[16:32, c0:c0 + F], in_=ntp3[:, 0:F])
                nc.scalar.dma_start(out=nt_d[48:64, c0:c0 + F], in_=ntp4[:, 0:F])
                nc.scalar.dma_start(out=nt_d[80:96, c0:c0 + F], in_=ntp5[:, 0:F])
                c0 += F

    nc.finalize()
    return nc


def kernel(inputs, h, c, n, params):
    bacc, mybir, tile, run_bass_kernel_spmd = _import_concourse()
    x = _np(inputs)
    h = _np(h)
    c = _np(c)
    B = x.shape[0]
    stride, s_dev, chunks, L4, L5 = _plan(B)
    wblob, bblob = _build_weights(params)
    in_maps = []
    for cix in range(NCORES):
        xh, c6, r4b, r5b = _host_prep_core(x, h, c, cix, B, stride, s_dev, L4, L5)
        in_maps.append(dict(xh=xh, c6=c6, r4b=r4b, r5b=r5b,
                            wblob=wblob, bblob=bblob))
    nc = _build_program(bacc, mybir, tile, s_dev, chunks, L4, L5)
    res = run_bass_kernel_spmd(nc, in_maps, list(range(NCORES))).results

    n_out = np.empty((NODES, B, H), np.float32)
    h_out = np.empty((NODES, B, H), np.float32)
    c_out = np.empty((NODES, B, H), np.float32)
    for cix in range(NCORES):
        b0 = cix * stride
        w = min(s_dev, B - b0)
        for j, i in enumerate(SIGMA):
            n_out[i, b0:b0 + w] = res[cix]["nt"][16 * j:16 * j + 16, :w].T.astype(np.float32)
            h_out[i, b0:b0 + w] = res[cix]["ht"][16 * j:16 * j + 16, :w].T.astype(np.float32)
            c_out[i, b0:b0 + w] = res[cix]["ct"][16 * j:16 * j + 16, :w].T.astype(np.float32)
    return n_out, h_out, c_out


# revision 5
# speedup vs baseline: 1.2592x; 1.0403x over previous
"""Trainium2 Bass kernel for nn_CausalLSTMCell (6-node causal LSTM cell over
batch 262144).  Self-contained: hardcodes shapes/sharding; host-side numpy
does layout transforms; 8 NeuronCores run an SPMD Tile kernel.

Layout strategy (feature-major): batch on the free dimension, per-node
features (16 each) stacked on partitions.  Dense node order SIGMA =
[0,3,1,4,2,5] puts leaf nodes at partition bases {0,32,64} (legal engine
bases) and internal nodes at {16,48,80} (bridged to base-0 tiles via
SBUF->SBUF DMA, which is partition-unrestricted).  The TF-faithful
child_r reshape scrambles batch rows; each core receives the exact
pre-gate source rows it needs (r4b/r5b buffers) and consumes them with
mod-2/mod-3 residue-class strided column ops.  Shard stride 32766 and
device width 32784 are multiples of 6 so residue phases are identical on
every core (single SPMD program).
"""
import sys
import numpy as np


def _import_concourse():
    for p in ("/opt/trn_rl_repo", "/root/.axon_site/_ro/trn_rl_repo"):
        if p not in sys.path:
            sys.path.insert(0, p)
    import concourse.bacc as bacc  # noqa: F401
    import concourse.mybir as mybir  # noqa: F401
    import concourse.tile as tile  # noqa: F401
    from concourse.bass_utils import run_bass_kernel_spmd  # noqa: F401
    return bacc, mybir, tile, run_bass_kernel_spmd


H = 16
NODES = 6
NCORES = 8
INPUT_IDX = [[0], [1], [2], [0, 3], [1, 4], [2, 5]]
CHILDREN = [0, 0, 0, 1, 2, 3]
SIGMA = [0, 3, 1, 4, 2, 5]          # dense row block j holds node SIGMA[j]
POS = [0, 2, 4, 1, 3, 5]            # node i lives at dense block POS[i]
F_MAIN = 1536

# wblob column offsets
WOFF = {"ig": 0, "fg": 96, "og": 192, "a": 288, "n2": 384,
        "n1r3": 480, "r4": 592, "r5": 656}
WCOLS = 752
# bias columns in bblob
BCOL = {"ig": 0, "fg": 1, "og": 2, "a": 3, "n2": 4, "n1r3": 5, "r4": 6, "r5": 7}


def _plan(B):
    stride = (B // NCORES // 6) * 6
    need = B - (NCORES - 1) * stride
    s_dev = ((need + 5) // 6) * 6
    n_main = s_dev // F_MAIN
    chunks = [F_MAIN] * n_main
    rem = s_dev - n_main * F_MAIN
    if rem:
        chunks.append(rem)
    L4 = s_dev // 2 + 2
    L5 = (2 + s_dev - 1) // 3 + 3
    return stride, s_dev, chunks, L4, L5


def _np(x):
    return np.asarray(x, dtype=np.float32)


def _build_weights(params):
    """wblob [128, 752] fp32 and bblob [128, 8] fp32 (see layout consts)."""
    wblob = np.zeros((128, WCOLS), np.float32)
    bblob = np.zeros((128, 8), np.float32)

    def put_role(dst_off, j, Wx, Wh, bx, bh, idx, i, w16=16):
        for d, xi in enumerate(idx):
            wblob[xi, dst_off + 16 * j: dst_off + 16 * j + w16] += _np(Wx)[d]
        wblob[6 + 16 * i: 6 + 16 * i + 16,
              dst_off + 16 * j: dst_off + 16 * j + w16] = _np(Wh)
        return _np(bx) + _np(bh)

    for j, i in enumerate(SIGMA):
        p = params[i]
        idx = INPUT_IDX[i]
        ifo_x, ifo_h = _np(p["ifo_x"]["W"]), _np(p["ifo_h"]["W"])
        ifo_bx, ifo_bh = _np(p["ifo_x"]["b"]), _np(p["ifo_h"]["b"])
        for role, sl in (("ig", slice(0, 16)), ("fg", slice(16, 32)),
                         ("og", slice(32, 48))):
            b = put_role(WOFF[role], j, ifo_x[:, sl], ifo_h[:, sl],
                         ifo_bx[sl], ifo_bh[sl], idx, i)
            bblob[16 * j:16 * j + 16, BCOL[role]] = b
        b = put_role(WOFF["a"], j, p["a_x"]["W"], p["a_h"]["W"],
                     p["a_x"]["b"], p["a_h"]["b"], idx, i)
        bblob[16 * j:16 * j + 16, BCOL["a"]] = b
        b = put_role(WOFF["n2"], j, p["n2_x"]["W"], p["n2_h"]["W"],
                     p["n2_x"]["b"], p["n2_h"]["b"], idx, i)
        bblob[16 * j:16 * j + 16, BCOL["n2"]] = b

    # n1r3 [102, 112]: r3@0, n1(node4)@32, n1(node5)@64, n1(node3)@96
    bblob[0:112, BCOL["n1r3"]] = -40.0
    p3 = params[3]
    for d, xi in enumerate(INPUT_IDX[3]):
        wblob[xi, WOFF["n1r3"] + 0: WOFF["n1r3"] + 16] += _np(p3["r_x"]["W"])[d]
    wblob[6 + 48:6 + 64, WOFF["n1r3"] + 0: WOFF["n1r3"] + 16] = _np(p3["r_h"]["W"])
    bblob[0:16, BCOL["n1r3"]] = _np(p3["r_x"]["b"]) + _np(p3["r_h"]["b"])
    for node_i, coff in ((4, 32), (5, 64), (3, 96)):
        p = params[node_i]
        for d, xi in enumerate(INPUT_IDX[node_i]):
            wblob[xi, WOFF["n1r3"] + coff: WOFF["n1r3"] + coff + 16] += \
                _np(p["n1_x"]["W"])[d]
        wblob[6 + 16 * node_i:6 + 16 * node_i + 16,
              WOFF["n1r3"] + coff: WOFF["n1r3"] + coff + 16] = _np(p["n1_h"]["W"])
        bblob[coff:coff + 16, BCOL["n1r3"]] = \
            _np(p["n1_x"]["b"]) + _np(p["n1_h"]["b"])

    # child weights: rows 0:2 = x cols INPUT_IDX[i]; rows 2:18 = h_i
    bblob[0:64, BCOL["r4"]] = -40.0
    bblob[0:96, BCOL["r5"]] = -40.0
    for node_i, key, nc_i in ((4, "r4", 2), (5, "r5", 3)):
        p = params[node_i]
        Wx, Wh = _np(p["r_x"]["W"]), _np(p["r_h"]["W"])
        bb = _np(p["r_x"]["b"]) + _np(p["r_h"]["b"])
        for g in range(nc_i):
            co = WOFF[key] + 32 * g
            wblob[0:2, co:co + 16] = Wx[:, 16 * g:16 * g + 16]
            wblob[2:18, co:co + 16] = Wh[:, 16 * g:16 * g + 16]
            bblob[32 * g:32 * g + 16, BCOL[key]] = bb[16 * g:16 * g + 16]
    return wblob, bblob


def _host_prep_core(x, h, c, cix, B, stride, s_dev, L4, L5):
    b0 = cix * stride
    cols = np.arange(b0, b0 + s_dev)
    valid = cols < B
    colsc = np.minimum(cols, B - 1)
    xh = np.zeros((102, s_dev), np.float32)
    xh[0:6, :] = np.where(valid, x[colsc, :].T, 0.0)
    for i in range(NODES):
        xh[6 + 16 * i:6 + 16 * i + 16, :] = np.where(valid, h[i, colsc, :].T, 0.0)
    c6 = np.zeros((96, s_dev), np.float16)
    for j, i in enumerate(SIGMA):
        c6[16 * j:16 * j + 16, :] = np.where(valid, c[i, colsc, :].T, 0.0).astype(np.float16)
    r4b = np.zeros((2, 18, L4), np.float32)
    for k in range(2):
        t0 = k * B + b0
        assert t0 % 2 == 0
        rho0 = t0 // 2
        rows = np.arange(rho0, rho0 + L4)
        rv = rows < B
        rc = np.minimum(rows, B - 1)
        r4b[k, 0:2, :] = np.where(rv, x[rc][:, INPUT_IDX[4]].T, 0.0)
        r4b[k, 2:18, :] = np.where(rv, h[4, rc, :].T, 0.0)
    r5b = np.zeros((3, 18, L5), np.float32)
    for k in range(3):
        t0 = k * B + b0
        delta = t0 % 3
        rho0 = (t0 - delta) // 3
        Lk = (delta + s_dev - 1) // 3 + 1
        assert Lk <= L5
        rows = np.arange(rho0, rho0 + Lk)
        rv = rows < B
        rc = np.minimum(rows, B - 1)
        r5b[k, 0:2, :Lk] = np.where(rv, x[rc][:, INPUT_IDX[5]].T, 0.0)
        r5b[k, 2:18, :Lk] = np.where(rv, h[5, rc, :].T, 0.0)
    return xh, c6, r4b, r5b


def _pieces(F):
    out = []
    p = 0
    while p < F:
        w = min(512, F - p)
        out.append((p, w))
        p += w
    return out


def _build_program(bacc, mybir, tile, s_dev, chunks, L4, L5):
    f32 = mybir.dt.float32
    f32r = mybir.dt.float32r
    bf16 = mybir.dt.float16
    Sig = mybir.ActivationFunctionType.Sigmoid
    Tanh = mybir.ActivationFunctionType.Tanh
    MUL = mybir.AluOpType.mult
    ADD = mybir.AluOpType.add

    nc = bacc.Bacc("TRN2", target_bir_lowering=False, debug=False,
                   num_devices=NCORES)
    xh_d = nc.dram_tensor("xh", [102, s_dev], f32r, kind="ExternalInput")
    c6_d = nc.dram_tensor("c6", [96, s_dev], bf16, kind="ExternalInput")
    r4_d = nc.dram_tensor("r4b", [2, 18, L4], f32r, kind="ExternalInput")
    r5_d = nc.dram_tensor("r5b", [3, 18, L5], f32r, kind="ExternalInput")
    wb_d = nc.dram_tensor("wblob", [128, WCOLS], f32r, kind="ExternalInput")
    bb_d = nc.dram_tensor("bblob", [128, 8], f32, kind="ExternalInput")
    nt_d = nc.dram_tensor("nt", [96, s_dev], bf16, kind="ExternalOutput")
    ht_d = nc.dram_tensor("ht", [96, s_dev], bf16, kind="ExternalOutput")
    ct_d = nc.dram_tensor("ct", [96, s_dev], bf16, kind="ExternalOutput")

    def _even(v):
        return v + (v % 2)

    FM = chunks[0]
    FHM, F3M = _even(FM // 2), _even(FM // 3 + 1)

    with tile.TileContext(nc) as tc:
        with tc.tile_pool(name="const", bufs=1) as cpool, \
             tc.tile_pool(name="io", bufs=3) as io, \
             tc.tile_pool(name="sig", bufs=2) as sg, \
             tc.tile_pool(name="work", bufs=2) as wk, \
             tc.tile_pool(name="narrow", bufs=2) as nr, \
             tc.tile_pool(name="st", bufs=2, space="PSUM") as psr, \
             tc.tile_pool(name="stc", bufs=1, space="PSUM") as psc:

            wb = cpool.tile([128, WCOLS], f32r)
            bb = cpool.tile([128, 8], f32)
            nc.sync.dma_start(out=wb, in_=wb_d[:, :])
            nc.sync.dma_start(out=bb, in_=bb_d[:, :])

            def role_matmul_act(XH, F, woff, rows, func, bcol, dst):
                st = psr.tile([112, FM], f32, tag="st")
                for p0, w in _pieces(F):
                    nc.tensor.matmul(st[0:rows, p0:p0 + w],
                                     wb[0:102, woff:woff + rows],
                                     XH[:, p0:p0 + w], start=True, stop=True)
                nc.scalar.activation(dst[:, 0:F], st[0:rows, 0:F], func,
                                     bias=bb[0:rows, bcol:bcol + 1], scale=1.0)

            c0 = 0
            for F in chunks:
                FH, F3, FT = _even(F // 2), _even(F // 3 + 1), F // 3
                XH = io.tile([102, FM], f32r, tag="xh")
                nc.sync.dma_start(out=XH[:, 0:F], in_=xh_d[:, c0:c0 + F])
                CP = io.tile([96, FM], bf16, tag="cp")
                nc.sync.dma_start(out=CP[:, 0:F], in_=c6_d[:, c0:c0 + F])
                R4 = []
                for k in range(2):
                    t = io.tile([18, FHM], f32r, tag=f"r4_{k}")
                    nc.sync.dma_start(out=t[:, 0:FH],
                                      in_=r4_d[k, :, c0 // 2:c0 // 2 + FH])
                    R4.append(t)
                R5 = []
                for k in range(3):
                    t = io.tile([18, F3M], f32r, tag=f"r5_{k}")
                    nc.sync.dma_start(out=t[:, 0:F3],
                                      in_=r5_d[k, :, c0 // 3:c0 // 3 + F3])
                    R5.append(t)

                IG = sg.tile([96, FM], bf16, tag="ig")
                FG = sg.tile([96, FM], bf16, tag="fg")
                OG = sg.tile([96, FM], bf16, tag="og")
                AT = sg.tile([96, FM], bf16, tag="at")
                N2 = sg.tile([96, FM], bf16, tag="n2")
                N1R = sg.tile([112, FM], bf16, tag="n1r")
                role_matmul_act(XH, F, WOFF["ig"], 96, Sig, BCOL["ig"], IG)
                role_matmul_act(XH, F, WOFF["fg"], 96, Sig, BCOL["fg"], FG)
                role_matmul_act(XH, F, WOFF["og"], 96, Sig, BCOL["og"], OG)
                role_matmul_act(XH, F, WOFF["a"], 96, Tanh, BCOL["a"], AT)
                role_matmul_act(XH, F, WOFF["n2"], 96, Sig, BCOL["n2"], N2)
                role_matmul_act(XH, F, WOFF["n1r3"], 112, Sig, BCOL["n1r3"], N1R)

                # child pre-gates: sigmoid in place in PSUM
                CR4 = []
                for k in range(2):
                    st = psc.tile([96, max(FHM, F3M)], f32, tag="stc")
                    for p0, w in _pieces(FH):
                        nc.tensor.matmul(st[0:64, p0:p0 + w],
                                         wb[0:18, WOFF["r4"]:WOFF["r4"] + 64],
                                         R4[k][:, p0:p0 + w],
                                         start=True, stop=True)
                    nc.scalar.activation(st[0:64, 0:FH], st[0:64, 0:FH], Sig,
                                         bias=bb[0:64, BCOL["r4"]:BCOL["r4"] + 1],
                                         scale=1.0)
                    CR4.append(st)
                CR5 = []
                for k in range(3):
                    st = psc.tile([96, max(FHM, F3M)], f32, tag="stc")
                    for p0, w in _pieces(F3):
                        nc.tensor.matmul(st[0:96, p0:p0 + w],
                                         wb[0:18, WOFF["r5"]:WOFF["r5"] + 96],
                                         R5[k][:, p0:p0 + w],
                                         start=True, stop=True)
                    nc.scalar.activation(st[0:96, 0:F3], st[0:96, 0:F3], Sig,
                                         bias=bb[0:96, BCOL["r5"]:BCOL["r5"] + 1],
                                         scale=1.0)
                    CR5.append(st)

                # horizontal pass (dense sigma order)
                CN = wk.tile([96, FM], bf16, tag="cn")
                T2 = wk.tile([96, FM], bf16, tag="t2")
                TC = wk.tile([96, FM], bf16, tag="tc")
                HN = wk.tile([96, FM], bf16, tag="hn")
                NT = wk.tile([96, FM], bf16, tag="nt")
                nc.vector.tensor_tensor(CN[:, 0:F], IG[:, 0:F], AT[:, 0:F], MUL)
                nc.vector.tensor_tensor(T2[:, 0:F], FG[:, 0:F], CP[:, 0:F], MUL)
                nc.vector.tensor_tensor(CN[:, 0:F], CN[:, 0:F], T2[:, 0:F], ADD)
                nc.scalar.activation(TC[:, 0:F], CN[:, 0:F], Tanh, scale=1.0)
                nc.vector.tensor_tensor(HN[:, 0:F], OG[:, 0:F], TC[:, 0:F], MUL)
                nc.vector.tensor_tensor(NT[:, 0:F], N2[:, 0:F], HN[:, 0:F], MUL)

                # narrow tiles (see module docstring for slot/base plan)
                SM1 = nr.tile([128, FM], bf16, tag="sm1")
                SM2 = nr.tile([128, FM], bf16, tag="sm2")
                SM3 = nr.tile([128, FM], bf16, tag="sm3")
                SM4 = nr.tile([64, FM], bf16, tag="sm4")
                SM5 = nr.tile([64, FM], bf16, tag="sm5")
                ntp3, p40, p50, p30 = SM1[0:16], SM1[32:48], SM1[64:80], SM1[96:112]
                T16a, p41, p51 = SM2[0:16], SM2[32:48], SM2[64:80]
                ntp4, p52 = SM3[0:16], SM3[64:80]
                T16b, ntp5 = SM4[0:16], SM4[32:48]
                T16c = SM5[32:48]

                # bridges: internal-node prelim n_new blocks to base-0/32 slots
                nc.sync.dma_start(out=ntp3[:, 0:F], in_=NT[16:32, 0:F])
                nc.sync.dma_start(out=ntp4[:, 0:F], in_=NT[48:64, 0:F])
                nc.sync.dma_start(out=ntp5[:, 0:F], in_=NT[80:96, 0:F])

                # node 3 (b-aligned child): r3 = sig(...) at N1R[0:16]
                nc.vector.tensor_tensor(p30[:, 0:F], N1R[0:16, 0:F],
                                        NT[0:16, 0:F], MUL)
                nc.vector.tensor_tensor(T16a[:, 0:F], N1R[96:112, 0:F],
                                        p30[:, 0:F], MUL)
                nc.gpsimd.tensor_tensor(ntp3[:, 0:F], ntp3[:, 0:F],
                                        T16a[:, 0:F], ADD)

                # node 4: classes mod 2; children n1 (dense@32), n2 (dense@64)
                for k, (pdst, nsrc) in enumerate(((p40, NT[32:48]),
                                                  (p41, NT[64:80]))):
                    for m2 in range(2):
                        nc.vector.tensor_tensor(
                            pdst[:, m2:F:2],
                            CR4[k][32 * m2:32 * m2 + 16, 0:FH],
                            nsrc[:, m2:F:2], MUL)
                nc.gpsimd.tensor_tensor(p40[:, 0:F], p40[:, 0:F],
                                        p41[:, 0:F], ADD)
                nc.gpsimd.tensor_tensor(T16b[:, 0:F], N1R[32:48, 0:F],
                                        p40[:, 0:F], MUL)
                nc.gpsimd.tensor_tensor(ntp4[:, 0:F], ntp4[:, 0:F],
                                        T16b[:, 0:F], ADD)

                # node 5: classes mod 3; children n0 (dense@0), n3f=ntp3, n4f=ntp4
                for k, (pdst, nsrc) in enumerate(((p50, NT[0:16]),
                                                  (p51, ntp3),
                                                  (p52, ntp4))):
                    for m3 in range(3):
                        g = (k + m3) % 3
                        joff = (k + m3) // 3
                        nc.vector.tensor_tensor(
                            pdst[:, m3:F:3],
                            CR5[k][32 * g:32 * g + 16, joff:joff + FT],
                            nsrc[:, m3:F:3], MUL)
                nc.gpsimd.tensor_tensor(p50[:, 0:F], p50[:, 0:F],
                                        p51[:, 0:F], ADD)
                nc.gpsimd.tensor_tensor(p50[:, 0:F], p50[:, 0:F],
                                        p52[:, 0:F], ADD)
                nc.gpsimd.tensor_tensor(T16c[:, 0:F], N1R[64:80, 0:F],
                                        p50[:, 0:F], MUL)
                nc.gpsimd.tensor_tensor(ntp5[:, 0:F], ntp5[:, 0:F],
                                        T16c[:, 0:F], ADD)

                # outputs
                nc.scalar.dma_start(out=ht_d[:, c0:c0 + F], in_=HN[:, 0:F])
                nc.scalar.dma_start(out=ct_d[:, c0:c0 + F], in_=CN[:, 0:F])
                nc.scalar.dma_start(out=nt_d[0:16, c0:c0 + F], in_=NT[0:16, 0:F])
                nc.scalar.dma_start(out=nt_d[32:48, c0:c0 + F], in_=NT[32:48, 0:F])
                nc.scalar.dma_start(out=nt_d[64:80, c0:c0 + F], in_=NT[64:80, 0:F])
                nc.scalar.dma_start(out=nt_d[16:32, c0:c0 + F], in_=ntp3[:, 0:F])
                nc.scalar.dma_start(out=nt_d[48:64, c0:c0 + F], in_=ntp4[:, 0:F])
                nc.scalar.dma_start(out=nt_d[80:96, c0:c0 + F], in_=ntp5[:, 0:F])
                c0 += F

    nc.finalize()
    return nc


def kernel(inputs, h, c, n, params):
    bacc, mybir, tile, run_bass_kernel_spmd = _import_concourse()
    x = _np(inputs)
    h = _np(h)
    c = _np(c)
    B = x.shape[0]
    stride, s_dev, chunks, L4, L5 = _plan(B)
    wblob, bblob = _build_weights(params)
    in_maps = []
    for cix in range(NCORES):
        xh, c6, r4b, r5b = _host_prep_core(x, h, c, cix, B, stride, s_dev, L4, L5)
        in_maps.append(dict(xh=xh, c6=c6, r4b=r4b, r5b=r5b,
                            wblob=wblob, bblob=bblob))
    nc = _build_program(bacc, mybir, tile, s_dev, chunks, L4, L5)
    res = run_bass_kernel_spmd(nc, in_maps, list(range(NCORES))).results

    n_out = np.empty((NODES, B, H), np.float32)
    h_out = np.empty((NODES, B, H), np.float32)
    c_out = np.empty((NODES, B, H), np.float32)
    for cix in range(NCORES):
        b0 = cix * stride
        w = min(s_dev, B - b0)
        for j, i in enumerate(SIGMA):
            n_out[i, b0:b0 + w] = res[cix]["nt"][16 * j:16 * j + 16, :w].T.astype(np.float32)
            h_out[i, b0:b0 + w] = res[cix]["ht"][16 * j:16 * j + 16, :w].T.astype(np.float32)
            c_out[i, b0:b0 + w] = res[cix]["ct"][16 * j:16 * j + 16, :w].T.astype(np.float32)
    return n_out, h_out, c_out
